# revision 1
# baseline (speedup 1.0000x reference)
"""Trainium2 Bass kernel for nn_ContextualAttention (sparse_attention).

Contract: kernel(**inputs) takes FULL numpy inputs and returns the FULL
[2, 256, 48, 48] float32 output. Internally shards across 8 NeuronCores as
(batch b in {0,1}) x (side l/r) x (position-half in {0,1}).

Design: scores in [l, p] layout (128-part l-tiles, no PE transposes), exp
without max-subtraction (logits <= ~50 for this input distribution), recon
on UNNORMALIZED exp with the softmax 1/denominator folded into a per-tile
scale after recon, on-device transpose-conv overlap-add into a bf16
[C, 26, 50] slab, bf16 mh/fp wrap-variants via triple DMA of one DRAM copy
(plus per-variant column memset), all matmuls bf16, double-buffered input
tiles so the next rep's DMA prefetches under this rep's compute.

Per-core device work for unit (b, side), half h (288 positions):
  scores[l, p] = sum_{ki,kj,ch} fp[ch, l+off] * mh[ch, p+off]   (18 matmuls)
  Eb[l, p] = exp(scores * (10*invd[l]))          (ACT per-partition scale)
  den[p] = sum_l Eb  (PE ones-matmul);  r = 1/den (DVE)
  po[cf, p] = sum_l rawT[l, cf] * Eb[l, p]       (cf = ch*2048+ij*128+c)
  slab[c, 2y+i, 2x+j] += po * r[p]               (DVE mul + strided add)
Host: downsample, pad/flatten images, rawT reorder+0.25 scale, feature-norm
inv, slab overlap-add across halves, cosine blend.
"""

import sys

for _p in ("/opt/trn_rl_repo", "/root/.axon_site/_ro/trn_rl_repo"):
    if _p not in sys.path:
        sys.path.append(_p)

import numpy as np
import ml_dtypes

BF16 = ml_dtypes.bfloat16

B, C, H, W = 2, 256, 48, 48
HD = WD = 24          # downsampled spatial
L = HD * WD           # 576 filter positions
PH = L // 2           # 288 positions per core (half)
CF = C * 16           # 4096 reconstruction features (ch, ij, c)
EPS_SUM = 2304 * 1e-4  # sum_k (f^2 + eps) = sumsq + K*eps
SCALE = 10.0
MHW = 14 * 24 + 2     # mh row length incl 1-elem zero guards
FPW = 26 * 24 + 2     # fp row length incl guards
SLABW = 26 * 50       # per-half output slab: rows 2y+i in 0..25, cols 2x+j

# l-tiles for the 576-long filter axis: 4x128 + 64
LT = [(0, 128), (128, 128), (256, 128), (384, 128), (512, 64)]

_CACHED = {}


def _build_nc(reps=1):
    from concourse import bacc, mybir
    from concourse.dt import dt
    from concourse.tile import TileContext

    f32 = dt.float32
    f32r = dt.float32r
    bf16 = dt.bfloat16

    nc = bacc.Bacc("TRN2", target_bir_lowering=False, debug=False,
                   num_devices=8)
    mh_d = nc.declare_dram_parameter("mh1", [C, MHW], bf16, isOutput=False)
    fp_d = nc.declare_dram_parameter("fp1", [C, FPW], bf16, isOutput=False)
    rawT_d = nc.declare_dram_parameter("rawT", [L, CF], bf16, isOutput=False)
    iv_d = nc.declare_dram_parameter("invd10", [L, 1], f32, isOutput=False)
    out_d = nc.declare_dram_parameter("out", [C, SLABW], bf16, isOutput=True)

    AF = mybir.ActivationFunctionType

    with TileContext(nc) as tc:
        with (
            tc.tile_pool(name="persist", bufs=1) as pp,
            tc.tile_pool(name="inbuf", bufs=2) as ib,
            tc.tile_pool(name="tmp", bufs=4) as sp,
            tc.tile_pool(name="ps_score", bufs=2, space="PSUM") as ps_s,
            tc.tile_pool(name="ps_den", bufs=1, space="PSUM") as ps_d,
            tc.tile_pool(name="ps_out", bufs=5, space="PSUM") as ps_o,
        ):
          for _rep in range(reps):
              # ---- persistent SBUF tensors + input DMAs ----
              # variant v: 0 -> kj=0 (col 23 zeroed), 1 -> middle, 2 -> kj=2
              mh = [[ib.tile([128, MHW], bf16, tag=f"mh{v}{i}", name=f"mh{v}{i}")
                     for i in range(2)] for v in range(3)]
              fp = [[ib.tile([128, FPW], bf16, tag=f"fp{v}{i}", name=f"fp{v}{i}")
                     for i in range(2)] for v in range(3)]
              rawT = [ib.tile([128, CF], bf16, tag=f"rawT{i}", name=f"rawT{i}")
                      for i in range(5)]
              Eb = [pp.tile([128, PH], bf16, tag=f"Eb{i}", name=f"Eb{i}")
                    for i in range(5)]
              iv = [pp.tile([128, 1], f32, tag=f"iv{i}", name=f"iv{i}")
                    for i in range(5)]
              slab = [pp.tile([128, SLABW], bf16, tag=f"slab{i}", name=f"slab{i}")
                      for i in range(2)]
              onesc = pp.tile([128, 1], bf16, tag="onesc", name="onesc")
              rrec = pp.tile([1, PH], f32, tag="rrec", name="rrec")
              rbc = pp.tile([128, PH], f32, tag="rbc", name="rbc")

              # 3 wrap-variants of each image: 3 DMA reads of the same DRAM
              # region (DMA has headroom; engine copies would stall PE at rep
              # boundaries), then zero the contaminated column per variant.
              # Order: v=1 (kj=1 scores run first) -> invd -> edge variants ->
              # rawT in column-major chunks so recon's first cf blocks have
              # all 5 l-tiles after 1/4 of the rawT traffic.
              for ch in range(2):
                  nc.scalar.dma_start(mh[1][ch][:, :],
                                      mh_d[ch * 128:(ch + 1) * 128, :])
                  nc.scalar.dma_start(fp[1][ch][:, :],
                                      fp_d[ch * 128:(ch + 1) * 128, :])
              for ch in range(2):
                  for v in (0, 2):
                      nc.scalar.dma_start(mh[v][ch][:, :],
                                          mh_d[ch * 128:(ch + 1) * 128, :])
                      nc.scalar.dma_start(fp[v][ch][:, :],
                                          fp_d[ch * 128:(ch + 1) * 128, :])
              # invd isn't read until the first exp (~5.5us in) — issue last
              for lt, (l0, lsz) in enumerate(LT):
                  nc.scalar.dma_start(iv[lt][0:lsz, :], iv_d[l0:l0 + lsz, :])
              # Small input DMAs ride the Activation DGE queue; the bulky
              # rawT stream keeps the SP queue so neither blocks the other.
              for c0 in range(0, CF, 1024):
                  for lt, (l0, lsz) in enumerate(LT):
                      nc.sync.dma_start(rawT[lt][0:lsz, c0:c0 + 1024],
                                        rawT_d[l0:l0 + lsz, c0:c0 + 1024])

              nc.vector.memset(onesc[:, :], 1.0)
              for ch in range(2):
                  nc.vector.memset(slab[ch][:, :], 0.0)

              for ch in range(2):
                  # kj=0 variant: zero col x=23; kj=2 variant: zero col x=0
                  # (guard offset 1: row r col x lives at 1 + r*24 + x)
                  for v, x in ((0, 23), (2, 0)):
                      nc.vector.memset(
                          mh[v][ch][:, 1 + x: 1 + x + 13 * 24 + 1: 24], 0.0)
                      nc.vector.memset(
                          fp[v][ch][:, 1 + x: 1 + x + 25 * 24 + 1: 24], 0.0)

              # ---- scores + exp, one l-tile at a time ----
              for lt, (l0, lsz) in enumerate(LT):
                  ps = ps_s.tile([128, PH], f32, tag="ps", name="ps")
                  k = 0
                  for kj in (1, 0, 2):
                      for ki in range(3):
                          for ch in range(2):
                              off = 24 * ki + kj
                              nc.tensor.matmul(
                                  ps[0:lsz, :],
                                  fp[kj][ch][:, l0 + off: l0 + off + lsz],
                                  mh[kj][ch][:, off: off + PH],
                                  start=(k == 0), stop=(k == 17))
                              k += 1
                  nc.scalar.activation(Eb[lt][0:lsz, :], ps[0:lsz, :], AF.Exp,
                                       scale=iv[lt][0:lsz, :])

              # ---- softmax denominator: den[p] = sum_l Eb; r = 1/den ----
              den = ps_d.tile([1, PH], f32, tag="den", name="den")
              for lt, (l0, lsz) in enumerate(LT):
                  nc.tensor.matmul(den[:, :], onesc[0:lsz, :], Eb[lt][0:lsz, :],
                                   start=(lt == 0), stop=(lt == 4))
              nc.vector.reciprocal(rrec[:, :], den[:, :])
              nc.gpsimd.partition_broadcast(rbc[:, :], rrec[:, :])

              # ---- reconstruction + on-chip overlap-add into slab ----
              # po is scaled by r and overlap-added AFTER recon (rbc is ready
              # well before the first po lands -> no PE stall); mul+add pairs
              # alternate DVE/Pool by cf parity. cf block order: ch-major so
              # slab[0] finishes mid-recon and its DMA overlaps the rest.
              for ch in range(2):
                  for ij in range(16):
                      i, j = ij >> 2, ij & 3
                      cf0 = ch * 2048 + ij * 128
                      po = ps_o.tile([128, PH], f32, tag="po", name="po")
                      for lt, (l0, lsz) in enumerate(LT):
                          nc.tensor.matmul(
                              po[:, :],
                              rawT[lt][0:lsz, cf0:cf0 + 128],
                              Eb[lt][0:lsz, :],
                              start=(lt == 0), stop=(lt == 4))
                      # GPSIMD TensorTensor is ~3us/op on HW (ucode) — keep
                      # all elementwise on DVE; Pool only does the broadcast.
                      tmp = sp.tile([128, PH], bf16, tag="tmp", name="tmp")
                      nc.vector.tensor_mul(tmp[:, :], po[:, :], rbc[:, :])
                      sv = slab[ch].rearrange(
                          "p (r c) -> p r c", r=26, c=50)[:, i:i + 23:2,
                                                          j:j + 47:2]
                      nc.vector.tensor_add(
                          sv, sv,
                          tmp.rearrange("p (y x) -> p y x", y=12, x=24))
                  nc.sync.dma_start(out_d[ch * 128:(ch + 1) * 128, :],
                                    slab[ch][:, :])

    nc.compile()
    return nc


def _prep_inputs(inputs):
    """Build the 8 per-core input maps from the full problem inputs."""
    left = np.asarray(inputs["left"], dtype=np.float32)
    right = np.asarray(inputs["right"], dtype=np.float32)
    mid = np.asarray(inputs["mid"], dtype=np.float32)
    sl = np.asarray(inputs["shortcut_l"], dtype=np.float32)
    sr = np.asarray(inputs["shortcut_r"], dtype=np.float32)

    m_ds = mid[:, :, ::2, ::2]
    f_ds = [left[:, :, ::2, ::2], right[:, :, ::2, ::2]]

    # mh: rows y in [-1, 12] (h=0) / [11, 24] (h=1), zero out-of-range,
    # flattened to 14*24 with 1-elem guards; single middle variant.
    mh1 = np.zeros((B, 2, C, MHW), np.float32)
    for b in range(B):
        for h in range(2):
            m14 = np.zeros((C, 14, 24), np.float32)
            if h == 0:
                m14[:, 1:14] = m_ds[b, :, 0:13]
            else:
                m14[:, 0:13] = m_ds[b, :, 11:24]
            mh1[b, h, :, 1:1 + 14 * 24] = m14.reshape(C, 14 * 24)
    # fp: rows y in [-1, 24]
    fp1 = np.zeros((B, 2, C, FPW), np.float32)
    invd10 = np.zeros((B, 2, L, 1), np.float32)
    for b in range(B):
        for side in range(2):
            f26 = np.zeros((C, 26, 24), np.float32)
            f26[:, 1:25] = f_ds[side][b]
            fp1[b, side, :, 1:1 + 26 * 24] = f26.reshape(C, 26 * 24)
            # host inv_denom: 3x3 window sums of per-pixel channel sumsq
            s = np.zeros((26, 26), np.float32)
            s[1:25, 1:25] = (f_ds[side][b] ** 2).sum(axis=0)
            d2 = np.zeros((24, 24), np.float32)
            for ki in range(3):
                for kj in range(3):
                    d2 += s[ki:ki + 24, kj:kj + 24]
            invd10[b, side] = (SCALE / np.sqrt(d2 + EPS_SUM)).reshape(L, 1)

    def raw_t(s):  # [C,48,48] -> [576, 4096] (l=(y,x), cf=(ch,ij,c)) * 0.25
        p = np.zeros((C, 50, 50), np.float32)
        p[:, 1:49, 1:49] = s
        st = p.strides
        v = np.lib.stride_tricks.as_strided(
            p, shape=(24, 24, C, 4, 4),
            strides=(2 * st[1], 2 * st[2], st[0], st[1], st[2]))
        # (y, x, C, i, j) -> (y, x, ch, i, j, c)
        v6 = v.reshape(24, 24, 2, 128, 4, 4).transpose(0, 1, 2, 4, 5, 3)
        return (np.ascontiguousarray(v6).reshape(L, CF) * 0.25)

    raws = [[raw_t(sl[b]), raw_t(sr[b])] for b in range(B)]

    in_maps = []
    for core in range(8):
        b, side, h = core >> 2, (core >> 1) & 1, core & 1
        in_maps.append({
            "mh1": mh1[b, h].astype(BF16),
            "fp1": fp1[b, side].astype(BF16),
            "rawT": raws[b][side].astype(BF16),
            "invd10": invd10[b, side],
        })
    return in_maps


def _postprocess(results):
    """results: list of 8 dicts with 'out' slab [256, 26*50] -> full output."""
    y = np.zeros((B, 2, C, 48, 48), np.float32)
    for b in range(B):
        for side in range(2):
            acc = np.zeros((C, 50, 50), np.float32)
            s0 = np.asarray(results[(b << 2) | (side << 1) | 0]["out"],
                            dtype=np.float32)
            s1 = np.asarray(results[(b << 2) | (side << 1) | 1]["out"],
                            dtype=np.float32)
            acc[:, 0:26] += s0.reshape(C, 26, 50)
            acc[:, 24:50] += s1.reshape(C, 26, 50)
            y[b, side] = acc[:, 1:49, 1:49]
    j = np.arange(W, dtype=np.float32)
    w = (0.5 * (np.cos(np.pi * j / (W - 1)) + 1.0)).reshape(1, 1, 1, W)
    return w * y[:, 0] + w[..., ::-1] * y[:, 1]


def _run(inputs, trace=False):
    from concourse.bass_utils import run_bass_kernel_spmd

    if "nc" not in _CACHED:
        _CACHED["nc"] = _build_nc()
    in_maps = _prep_inputs(inputs)
    res = run_bass_kernel_spmd(_CACHED["nc"], in_maps, list(range(8)),
                               trace=trace)
    return _postprocess(res.results), res


def kernel(**inputs):
    out, _ = _run(inputs)
    return out



# revision 15
# speedup vs baseline: 4.2304x; 4.2304x over previous
"""Trainium2 Bass kernel for nn_ContextualAttention (sparse_attention).

Contract: kernel(**inputs) takes FULL numpy inputs and returns the FULL
[2, 256, 48, 48] float32 output. Internally shards across 8 NeuronCores as
(batch b in {0,1}) x (side l/r) x (position-half in {0,1}).

Design notes (v2, flat-26 layout):
  * Images are flattened with row stride 26 (1 left + 1 right guard col per
    row), so a 3x3 tap (ki,kj) is a single offset t = 26*ki + kj into the
    SAME tile for every tap -- no kj wrap-variants, no variant DMAs/copies/
    memsets. Filter index l' = 26*yl + xl is sparse (guard rows excluded
    downstream); position index p stays dense via a 3D moving-operand view
    mh[ch, ki:ki+12, kj:kj+24].
  * scores[l',p] accumulate 18 bf16 matmuls per 128-row l'-tile (5 tiles);
    exp via ACT with per-partition scale 10/sqrt(den_l') (0 at guard rows);
    den[p] interleaved per-tile as a ones-pattern matmul (pattern zeroes
    guard rows); 1/den folded into Eb (5 DVE muls) instead of per-block
    po scaling; the first HYB recon blocks run on raw Eb + po*rbc so the
    PE never waits for the reciprocal/broadcast.
  * recon po[cf,p] = sum_l rawT[l',cf]*EbN[l',p] (guard rawT rows are 0),
    overlap-added into a bf16 [C,26,50] slab on DVE, slab DMA'd per ch-half.
  * DMA: 9 instructions total (each costs ~625ns on the serialized HWDGE):
    mh-ch0, fp-ch0, mh-ch1(+ones), fp-ch1, iv, 4 rawT chunks; outputs ride
    the ACT queue so rep N+1 input DMAs are not blocked behind them.
Host: downsample, flat-26 packing, inv-denominator, rawT reorder + 0.25
scale + l' zero-stuffing, slab overlap-add across halves, cosine blend.
"""

import sys

for _p in ("/opt/trn_rl_repo", "/root/.axon_site/_ro/trn_rl_repo"):
    if _p not in sys.path:
        sys.path.append(_p)

import numpy as np
import ml_dtypes

BF16 = ml_dtypes.bfloat16

B, C, H, W = 2, 256, 48, 48
HD = WD = 24          # downsampled spatial
PH = 12 * 24          # 288 positions per core (half)
LP = 640              # padded filter rows: 5*128, l' = 26*yl + xl
NLT = 5               # l'-tiles of 128
CF = C * 16           # 4096 reconstruction features (ch, ij, c)
EPS_SUM = 2304 * 1e-4
SCALE = 10.0
MW = 14 * 26          # mh26 block: rows y-window(14) x 26 cols
FW = 26 * 26 + 28     # fp26 block: 676 flat + pad to 704 (l'+t <= 693)
SLABW = 26 * 50       # per-half output slab
RCH = 4               # rawT DMA chunks (cf-major, 1024 cols each)
HYB = 6               # recon blocks run on raw Eb before EbN is ready
NWARM = 9             # PE clock warm-up matmuls

_CACHED = {}


def _build_nc(reps=1):
    from concourse import bacc, mybir
    from concourse.dt import dt
    from concourse.tile import TileContext

    f32 = dt.float32
    bf16 = dt.bfloat16
    AF = mybir.ActivationFunctionType

    nc = bacc.Bacc("TRN2", target_bir_lowering=False, debug=False,
                   num_devices=8)
    a_d = nc.declare_dram_parameter("a", [128, 2 * MW + 5], bf16,
                                    isOutput=False)
    b_d = nc.declare_dram_parameter("b", [128, 2 * FW], bf16, isOutput=False)
    c_d = nc.declare_dram_parameter("c", [128, NLT], f32, isOutput=False)
    r_d = nc.declare_dram_parameter("r", [128, NLT * CF], bf16,
                                    isOutput=False)
    out_d = nc.declare_dram_parameter("out", [C, SLABW], bf16, isOutput=True)

    with TileContext(nc) as tc:
        with (
            tc.tile_pool(name="persist", bufs=1) as pp,
            tc.tile_pool(name="inbuf", bufs=2) as ib,
            tc.tile_pool(name="tmp", bufs=4) as sp,
            tc.tile_pool(name="ps_mm", bufs=7, space="PSUM") as ps_m,
            tc.tile_pool(name="ps_den", bufs=1, space="PSUM") as ps_d,
        ):
          for _rep in range(reps):
              mh = ib.tile([128, 2 * MW + 5], bf16, tag="mh", name="mh")
              fp = ib.tile([128, 2 * FW], bf16, tag="fp", name="fp")
              iv = ib.tile([128, NLT], f32, tag="iv", name="iv")
              rw = ib.tile([128, NLT * CF], bf16, tag="rw", name="rw")
              Eb = [pp.tile([128, PH], bf16, tag=f"Eb{i}", name=f"Eb{i}")
                    for i in range(NLT)]
              EbN = [pp.tile([128, PH], bf16, tag=f"EbN{i}", name=f"EbN{i}")
                     for i in range(NLT)]
              slab = [pp.tile([128, SLABW], bf16, tag=f"slab{i}",
                              name=f"slab{i}") for i in range(2)]
              rrec = pp.tile([1, PH], bf16, tag="rrec", name="rrec")
              rbc = pp.tile([128, PH], bf16, tag="rbc", name="rbc")

              # ---- input DMAs: first-needed first, all on the SP queue so
              # output DMAs (ACT queue) never block the next rep's inputs.
              nc.sync.dma_start(fp[:, 0:FW], b_d[:, 0:FW])
              nc.sync.dma_start(mh[:, 0:MW], a_d[:, 0:MW])
              nc.sync.dma_start(fp[:, FW:2 * FW], b_d[:, FW:2 * FW])
              nc.sync.dma_start(mh[:, MW:2 * MW + 5], a_d[:, MW:2 * MW + 5])
              nc.sync.dma_start(iv[:, :], c_d[:, :])
              # rawT chunk-major DRAM layout: chunk k holds cols
              # [k*5120:(k+1)*5120] = (5 l'-tiles) x (1024 cf cols).
              csz = NLT * CF // RCH
              for k in range(RCH):
                  nc.sync.dma_start(rw[:, k * csz:(k + 1) * csz],
                                    r_d[:, k * csz:(k + 1) * csz])

              for ch in range(2):
                  nc.vector.memset(slab[ch][:, :], 0.0)

              # PE warm-up: the tensor engine ramps 0.65 -> 2.4 GHz over
              # ~3us of continuous execution. Run throwaway matmuls on a
              # locally-memset tile while the first input DMAs are in
              # flight so real matmuls start at full clock.
              if _rep == 0:
                  dm = pp.tile([128, PH], bf16, tag="dm", name="dm")
                  nc.gpsimd.memset(dm[:, :], 0.0)
                  # trigger the ACT Exp table load early (it takes ~1.3us);
                  # output goes to scratch so dm readers don't wait on it
                  dsc = sp.tile([1, 2], bf16, tag="dsc", name="dsc")
                  nc.scalar.activation(dsc[:, :], dm[0:1, 0:2], AF.Exp)
                  for wi in range(NWARM):
                      wp = ps_m.tile([128, PH], f32, tag="ps", name="ps")
                      nc.tensor.matmul(wp[:, :], dm[:, 0:128], dm[:, :],
                                       start=True, stop=True)

              # 3D tap views: mh3[ch][:, ki:ki+12, kj:kj+24]
              mh3 = [mh[:, ch * MW:(ch + 1) * MW].rearrange(
                  "p (r c) -> p r c", r=14, c=26) for ch in range(2)]

              # ---- scores + exp + den, one l'-tile at a time ----
              den = ps_d.tile([1, PH], f32, tag="den", name="den")
              for lt in range(NLT):
                  l0 = lt * 128
                  ps = ps_m.tile([128, PH], f32, tag="ps", name="ps")
                  k = 0
                  for ch in range(2):
                      for ki in range(3):
                          for kj in range(3):
                              t = 26 * ki + kj
                              nc.tensor.matmul(
                                  ps[:, :],
                                  fp[:, ch * FW + l0 + t:
                                     ch * FW + l0 + t + 128],
                                  mh3[ch][:, ki:ki + 12, kj:kj + 24],
                                  start=(k == 0), stop=(k == 17))
                              k += 1
                  nc.scalar.activation(Eb[lt][:, :], ps[:, :], AF.Exp,
                                       scale=iv[:, lt:lt + 1])
                  # ones-pattern excludes guard/pad l' rows from the denom
                  nc.tensor.matmul(den[:, :], mh[:, 2 * MW + lt:2 * MW + lt + 1],
                                   Eb[lt][:, :],
                                   start=(lt == 0), stop=(lt == NLT - 1))

              with nc.allow_low_precision(reason="1/den in bf16; ~0.4% on"
                                          " softmax scale, within tolerance"):
                  nc.vector.reciprocal(rrec[:, :], den[:, :])
              nc.gpsimd.partition_broadcast(rbc[:, :], rrec[:, :])
              for lt in range(NLT):
                  nc.vector.tensor_mul(EbN[lt][:, :], Eb[lt][:, :], rbc[:, :])

              # ---- reconstruction + overlap-add into slab ----
              for ch in range(2):
                  # i-order (0,2,1,3): even slab rows are final after the
                  # first 8 blocks, odd rows after all 16 -> the output DMA
                  # splits even/odd so only half the bytes trail the last add
                  for bj, (i, j) in enumerate(
                          (i, j) for i in (0, 2, 1, 3) for j in range(4)):
                      bi = ch * 16 + bj
                      ij = i * 4 + j
                      cf0 = ch * 2048 + ij * 128
                      ck, co = cf0 // 1024, cf0 % 1024
                      po = ps_m.tile([128, PH], f32, tag="ps", name="po")
                      esrc = Eb if bi < HYB else EbN
                      for lt in range(NLT):
                          col = ck * csz + lt * 1024 + co
                          nc.tensor.matmul(po[:, :], rw[:, col:col + 128],
                                           esrc[lt][:, :],
                                           start=(lt == 0), stop=(lt == NLT - 1))
                      sv = slab[ch].rearrange(
                          "p (r c) -> p r c", r=26, c=50)[:, i:i + 23:2,
                                                          j:j + 47:2]
                      if bi < HYB:
                          tmp = sp.tile([128, PH], bf16, tag="tmp", name="tmp")
                          nc.vector.tensor_mul(tmp[:, :], po[:, :], rbc[:, :])
                          nc.vector.tensor_add(
                              sv, sv,
                              tmp.rearrange("p (y x) -> p y x", y=12, x=24))
                      else:
                          nc.vector.tensor_add(
                              sv, sv,
                              po.rearrange("p (y x) -> p y x", y=12, x=24))
                      if bj == 7 or bj == 15:
                          par = 0 if bj == 7 else 1
                          dv = out_d[ch * 128:(ch + 1) * 128, :].rearrange(
                              "p (r c) -> p r c", r=26, c=50)[:, par:26:2, :]
                          sl = slab[ch].rearrange(
                              "p (r c) -> p r c", r=26, c=50)[:, par:26:2, :]
                          # final (odd) piece on SP: shorter DGE delay
                          if bj == 15:
                              nc.sync.dma_start(dv, sl)
                          else:
                              nc.scalar.dma_start(dv, sl)

    nc.compile()
    return nc


def _prep_inputs(inputs):
    """Build the 8 per-core input maps from the full problem inputs."""
    left = np.asarray(inputs["left"], dtype=np.float32)
    right = np.asarray(inputs["right"], dtype=np.float32)
    mid = np.asarray(inputs["mid"], dtype=np.float32)
    sl = np.asarray(inputs["shortcut_l"], dtype=np.float32)
    sr = np.asarray(inputs["shortcut_r"], dtype=np.float32)

    m_ds = mid[:, :, ::2, ::2]
    f_ds = [left[:, :, ::2, ::2], right[:, :, ::2, ::2]]

    # validity of padded filter index l' = 26*yl + xl
    lp = np.arange(LP)
    valid = (lp % 26 < 24) & (lp // 26 < 24)
    ones5 = valid.astype(np.float32).reshape(NLT, 128).T  # [128, 5]

    # mh26: [C,14,26] guard-framed mid rows per half -> a = [mh0|mh1|ones5]
    a_all = np.zeros((B, 2, 128, 2 * MW + 5), np.float32)
    for b in range(B):
        for h in range(2):
            m14 = np.zeros((C, 14, 26), np.float32)
            if h == 0:
                m14[:, 1:14, 1:25] = m_ds[b, :, 0:13]
            else:
                m14[:, 0:13, 1:25] = m_ds[b, :, 11:24]
            m14 = m14.reshape(2, 128, MW)
            a_all[b, h, :, 0:MW] = m14[0]
            a_all[b, h, :, MW:2 * MW] = m14[1]
            a_all[b, h, :, 2 * MW:] = ones5

    # fp26: [C,26,26] guard-framed features -> b = [fp0|fp1], 704-padded
    b_all = np.zeros((B, 2, 128, 2 * FW), np.float32)
    c_all = np.zeros((B, 2, 128, NLT), np.float32)
    for b in range(B):
        for side in range(2):
            f26 = np.zeros((C, 26, 26), np.float32)
            f26[:, 1:25, 1:25] = f_ds[side][b]
            f26 = f26.reshape(2, 128, 676)
            b_all[b, side, :, 0:676] = f26[0]
            b_all[b, side, :, FW:FW + 676] = f26[1]
            # inv denominator: 3x3 window sums of per-pixel channel sumsq
            s = np.zeros((26, 26), np.float32)
            s[1:25, 1:25] = (f_ds[side][b] ** 2).sum(axis=0)
            d2 = np.zeros((24, 24), np.float32)
            for ki in range(3):
                for kj in range(3):
                    d2 += s[ki:ki + 24, kj:kj + 24]
            ivl = SCALE / np.sqrt(d2 + EPS_SUM)  # [24, 24] over (yl, xl)
            ivp = np.zeros(LP, np.float32)
            ivp[valid] = ivl.reshape(-1)
            c_all[b, side] = ivp.reshape(NLT, 128).T

    def raw_t(s):  # [C,48,48] -> [128, 5*4096] chunk-major l'-layout * 0.25
        p = np.zeros((C, 50, 50), np.float32)
        p[:, 1:49, 1:49] = s
        st = p.strides
        v = np.lib.stride_tricks.as_strided(
            p, shape=(24, 24, C, 4, 4),
            strides=(2 * st[1], 2 * st[2], st[0], st[1], st[2]))
        # (y, x, C, i, j) -> (y, x, ch, i, j, c) -> [576, 4096]
        v6 = v.reshape(24, 24, 2, 128, 4, 4).transpose(0, 1, 2, 4, 5, 3)
        r576 = np.ascontiguousarray(v6).reshape(576, CF) * 0.25
        rlp = np.zeros((LP, CF), np.float32)
        rlp[valid] = r576
        # [5, 128, 4096] -> [128, chunks(4), lt(5), 1024] -> [128, 20480]
        r5 = rlp.reshape(NLT, 128, CF).transpose(1, 0, 2)
        return r5.reshape(128, NLT, RCH, CF // RCH).transpose(
            0, 2, 1, 3).reshape(128, NLT * CF)

    raws = [[raw_t(sl[b]), raw_t(sr[b])] for b in range(B)]

    in_maps = []
    for core in range(8):
        b, side, h = core >> 2, (core >> 1) & 1, core & 1
        in_maps.append({
            "a": a_all[b, h].astype(BF16),
            "b": b_all[b, side].astype(BF16),
            "c": c_all[b, side],
            "r": raws[b][side].astype(BF16),
        })
    return in_maps


def _postprocess(results):
    """results: list of 8 dicts with 'out' slab [256, 26*50] -> full output."""
    y = np.zeros((B, 2, C, 48, 48), np.float32)
    for b in range(B):
        for side in range(2):
            acc = np.zeros((C, 50, 50), np.float32)
            s0 = np.asarray(results[(b << 2) | (side << 1) | 0]["out"],
                            dtype=np.float32)
            s1 = np.asarray(results[(b << 2) | (side << 1) | 1]["out"],
                            dtype=np.float32)
            acc[:, 0:26] += s0.reshape(C, 26, 50)
            acc[:, 24:50] += s1.reshape(C, 26, 50)
            y[b, side] = acc[:, 1:49, 1:49]
    j = np.arange(W, dtype=np.float32)
    w = (0.5 * (np.cos(np.pi * j / (W - 1)) + 1.0)).reshape(1, 1, 1, W)
    return w * y[:, 0] + w[..., ::-1] * y[:, 1]


def _run(inputs, trace=False):
    from concourse.bass_utils import run_bass_kernel_spmd

    if "nc" not in _CACHED:
        _CACHED["nc"] = _build_nc()
    in_maps = _prep_inputs(inputs)
    res = run_bass_kernel_spmd(_CACHED["nc"], in_maps, list(range(8)),
                               trace=trace)
    return _postprocess(res.results), res


def kernel(**inputs):
    out, _ = _run(inputs)
    return out
